# revision 1
# baseline (speedup 1.0000x reference)
"""AuthPct metric kernel for 8 Trainium2 NeuronCores.

Sharding: real_stats rows are sharded across the 8 cores (1536 each,
the i/rhs side); gen and (host-rotated) real columns are the lhs side.
Each core computes coarse (fp8) tiles

    X[j, i] = 2*f_j[0:254].r_i[0:254] - |r_i|^2     [j-tile, 1536 i]

with ONE fp8 DoubleRow matmul (K=256) per 512-i PSUM bank: k-rows
0..253 carry features, k-rows 254/255 carry a scaled hi/lo fp8 split
of -|r_i|^2.  No augmented matmul, no bf16 pass, no PSUM->SBUF copy.

Per j-slot the core processes one gen j-tile and one real j-tile (the
real side uses the full 96-tile rotation, so every ordered real pair
appears once as a row-perspective; no partition reduce is needed):

 - gen: DVE segmented tensor_reduce (max) straight out of the 3-bank
   PSUM tile -> per-128-block coarse maxima, o_genv [128, 96*12].
 - real: ScalarE smooth-min straight out of each PSUM bank:
   ACTIVATE Exp with scale 1/T, per-partition bias (C0-|r_q|^2)/T and
   accum_out -> acc = sum_i exp((C0 - d^2(q,i))/T), one f32 per
   (row q, 512-block), o_reals [128, 96*3].  -T*ln(acc)+C0 is a
   smooth lower bound of the block min d^2 (within T*ln(n_eff)).

The Pool engine (5.4us per partition-reduce, the old co-bottleneck) is
not used at all.

The host min-combines the coarse partials, then refines exactly (f32
gemms over candidate blocks within a noise margin of each coarse
winner; the real diagonal block is always refined) to recover the
exact gen argmin + d1 and exact realNN at the used indices.  Device
noise (fp8, dropped dims, smooth-min slack) only widens the margins;
the returned values are exact fp32.
"""

import numpy as np

N = 12288
D = 256
DE = 254                     # feature dims carried on device
NCORES = 8
SHARD = N // NCORES          # 1536 rows per core
JTILE = 128                  # j columns per tile (PSUM partitions)
NJT = N // JTILE             # 96 gen j-tiles
RJT = 96                     # real j-tiles: full rotation c..c+7
NT = 512                     # i elements per matmul (PSUM bank)
NIT = SHARD // NT            # 3 i-tiles
NBLK = SHARD // JTILE        # 12 i-blocks of 128 per core
NSB = N // NT                # 24 global 512-blocks
NSCALE = 4.0                 # fp8 norm-row scale: rows carry -|r|^2/NSCALE
MARGIN_G = 12.0              # d^2 margin for gen argmin refinement
MARGIN_R = 20.0              # d^2 margin for realNN refinement (smooth)
MARGIN_RV = 12.0             # d^2 margin for realNN refinement (DVE exact)
TSM = 4.0                    # smooth-min temperature
C0 = 250.0                   # smooth-min shift (~min real-real NN d^2)
NVR = 16                     # real tiles scanned by the DVE instead of ScalarE
VSLOT = [jt for jt in range(NJT)
         if (jt * NVR) // NJT != ((jt + 1) * NVR) // NJT]

_cached_nc = None


def _build_nc():
    import concourse.mybir as mybir
    from concourse import bacc
    from concourse.tile import TileContext

    f32 = mybir.dt.float32
    fp8 = mybir.dt.float8e4

    nc = bacc.Bacc("TRN2", target_bir_lowering=False, debug=False,
                   num_devices=NCORES)

    # DoubleRow layouts: [p, (tile, t, col)] with K row = t*128 + p
    colg8 = nc.dram_tensor("colg8", [128, NJT * 2 * JTILE], fp8,
                           kind="ExternalInput")
    colr8 = nc.dram_tensor("colr8", [128, RJT * 2 * JTILE], fp8,
                           kind="ExternalInput")
    rhs8 = nc.dram_tensor("rhs8", [128, 2 * SHARD], fp8,
                          kind="ExternalInput")
    biasr = nc.dram_tensor("biasr", [128, RJT], f32,
                           kind="ExternalInput")

    o_genv = nc.dram_tensor("o_genv", [128, NJT * NBLK], f32,
                            kind="ExternalOutput")
    o_reals = [nc.dram_tensor(f"o_reals{k}", [128, RJT], f32,
                              kind="ExternalOutput") for k in range(NIT)]
    o_realv = nc.dram_tensor("o_realv", [128, NVR * NBLK], f32,
                             kind="ExternalOutput")

    with TileContext(nc) as tc:
        with (
            tc.tile_pool(name="const", bufs=1) as constp,
            tc.tile_pool(name="lhs", bufs=8) as lhsp,
            tc.tile_pool(name="junk", bufs=6) as junkp,
            tc.tile_pool(name="outb", bufs=1) as outp,
            tc.tile_pool(name="psg", bufs=2, space="PSUM") as psgp,
            tc.tile_pool(name="psr", bufs=2, space="PSUM") as psrp,
        ):
            rhs8_sb = constp.tile([128, 2 * SHARD], fp8)
            nc.sync.dma_start(out=rhs8_sb[:, 0:NT], in_=rhs8[:, 0:NT])
            nc.sync.dma_start(out=rhs8_sb[:, SHARD:SHARD + NT],
                              in_=rhs8[:, SHARD:SHARD + NT])
            biasr_sb = constp.tile([128, RJT], f32)
            nc.sync.dma_start(out=biasr_sb[:, :], in_=biasr[:, :])

            warm = junkp.tile([128, NT], f32, tag="junk")
            nc.scalar.activation(
                out=warm[:, 0:1], in_=biasr_sb[:, 0:1],
                func=mybir.ActivationFunctionType.Exp)

            genv = outp.tile([128, NJT * NBLK], f32)
            racc0 = outp.tile([128, RJT], f32)
            racc1 = outp.tile([128, RJT], f32)
            racc2 = outp.tile([128, RJT], f32)
            racc = [racc0, racc1, racc2]
            realv = outp.tile([128, NVR * NBLK], f32)
            vslot_idx = {jt: k for k, jt in enumerate(VSLOT)}

            def rhs_ap(io):
                return rhs8_sb[:, :].rearrange(
                    "p (t i) -> p t i", t=2)[:, :, io:io + NT]

            for jt in range(NJT):
                lhs_g = lhsp.tile([128, 2 * JTILE], fp8, tag="lhs_g")
                nc.sync.dma_start(
                    out=lhs_g[:, :],
                    in_=colg8[:, jt * 2 * JTILE:(jt + 1) * 2 * JTILE],
                )
                lhs_r = lhsp.tile([128, 2 * JTILE], fp8, tag="lhs_r")
                # real lhs loads ride the otherwise-idle GpSimd DMA queue so
                # the two per-slot loads issue in parallel, not serially
                nc.gpsimd.dma_start(
                    out=lhs_r[:, :],
                    in_=colr8[:, jt * 2 * JTILE:(jt + 1) * 2 * JTILE],
                )
                if jt == 0:
                    # remaining const slices, behind jt0's critical loads
                    for it0 in range(1, NIT):
                        io0 = it0 * NT
                        nc.sync.dma_start(out=rhs8_sb[:, io0:io0 + NT],
                                          in_=rhs8[:, io0:io0 + NT])
                        nc.sync.dma_start(
                            out=rhs8_sb[:, SHARD + io0:SHARD + io0 + NT],
                            in_=rhs8[:, SHARD + io0:SHARD + io0 + NT])

                # gen: one 3-bank PSUM tile, reduced PSUM-direct by the DVE.
                # Gen matmuls are issued one bank ahead of the real ones so
                # the wide gen reduce can start a matmul earlier.
                ps_g = psgp.tile([128, SHARD], f32, tag="ps_g")
                gen_mm = []
                for it in range(NIT):
                    io = it * NT
                    gen_mm.append(lambda io=io: nc.tensor.matmul(
                        out=ps_g[:, io:io + NT],
                        lhsT=lhs_g[:, :].rearrange("p (t j) -> p t j", t=2),
                        rhs=rhs_ap(io),
                        start=True, stop=True,
                        perf_mode=mybir.MatmulPerfMode.DoubleRow,
                    ))
                gen_mm[0]()
                for it in range(NIT):
                    io = it * NT
                    if it + 1 < NIT:
                        gen_mm[it + 1]()
                    ps_r = psrp.tile([128, NT], f32, tag="ps_r")
                    nc.tensor.matmul(
                        out=ps_r[:, :],
                        lhsT=lhs_r[:, :].rearrange("p (t j) -> p t j", t=2),
                        rhs=rhs_ap(io),
                        start=True, stop=True,
                        perf_mode=mybir.MatmulPerfMode.DoubleRow,
                    )
                    if jt in vslot_idx:
                        # DVE path: per-128-block maxima of X (PSUM-direct);
                        # the per-q |r_q|^2 shift is applied on the host
                        vo = (vslot_idx[jt] * NIT + it) * 4
                        nc.vector.tensor_reduce(
                            out=realv[:, vo:vo + 4],
                            in_=ps_r[:, :].rearrange("p (b x) -> p b x", b=4),
                            axis=mybir.AxisListType.X,
                            op=mybir.AluOpType.max)
                    else:
                        # acc[q] = sum_i exp((X - |r_q|^2 + C0)/T), PSUM-direct
                        junk = junkp.tile([128, NT], f32, tag="junk")
                        nc.scalar.activation(
                            out=junk[:, :],
                            in_=ps_r[:, :],
                            func=mybir.ActivationFunctionType.Exp,
                            bias=biasr_sb[:, jt:jt + 1],
                            scale=1.0 / TSM,
                            accum_out=racc[it][:, jt:jt + 1],
                        )

                # per-128-block maxima of X, straight out of PSUM
                nc.vector.tensor_reduce(
                    out=genv[:, jt * NBLK:(jt + 1) * NBLK],
                    in_=ps_g[:, :].rearrange("p (b x) -> p b x", b=NBLK),
                    axis=mybir.AxisListType.X,
                    op=mybir.AluOpType.max)

            nc.sync.dma_start(out=o_genv[:, :], in_=genv[:, :])
            for k in range(NIT):
                nc.sync.dma_start(out=o_reals[k][:, :], in_=racc[k][:, :])
            nc.sync.dma_start(out=o_realv[:, :], in_=realv[:, :])

    nc.compile()
    return nc


def _dr_pack(featT, f8, norm_hi, norm_lo):
    """[256-K, C] f32 -> fp8 DoubleRow [128, (tile, t, col)] layout.

    Rows 254/255 get the scaled norm hi/lo (rhs side) or the NSCALE
    constant (lhs side).
    """
    Dd, C = featT.shape
    assert Dd == D and C % JTILE == 0
    nt_ = C // JTILE
    a = featT.copy()
    a[DE] = norm_hi if norm_hi is not None else NSCALE
    a[DE + 1] = norm_lo if norm_lo is not None else NSCALE
    out = (a.reshape(2, 128, nt_, JTILE).transpose(1, 2, 0, 3)
           .reshape(128, nt_ * 2 * JTILE))
    return np.ascontiguousarray(out).astype(f8)


def kernel(real_stats, gen_stats, _trace=False):
    import ml_dtypes
    from concourse.bass_utils import run_bass_kernel_spmd

    f8 = ml_dtypes.float8_e4m3
    global _cached_nc
    real = np.ascontiguousarray(np.asarray(real_stats, dtype=np.float32))
    gen = np.ascontiguousarray(np.asarray(gen_stats, dtype=np.float32))

    realT = np.ascontiguousarray(real.T)                  # [D, N]
    genT = np.ascontiguousarray(gen.T)
    b2 = np.sum(real.astype(np.float64) ** 2, axis=1).astype(np.float32)
    a2g = np.sum(gen.astype(np.float64) ** 2, axis=1).astype(np.float32)

    colg8_np = _dr_pack(genT, f8, None, None)

    in_maps = []
    for c in range(NCORES):
        sl = slice(c * SHARD, (c + 1) * SHARD)
        t = -b2[sl] / NSCALE
        hi = t.astype(f8)
        lo = (t - hi.astype(np.float32)).astype(f8)
        rhs_full = 2.0 * realT[:, sl]
        rhs_full[DE] = hi.astype(np.float32)
        rhs_full[DE + 1] = lo.astype(np.float32)
        rhs8_np = np.ascontiguousarray(
            rhs_full.reshape(2, 128, SHARD).transpose(1, 0, 2)
            .reshape(128, 2 * SHARD)).astype(f8)
        colr_rot = np.roll(realT, -c * SHARD, axis=1)     # full rotation
        colr8_np = _dr_pack(colr_rot, f8, None, None)
        b2rot = np.roll(b2, -c * SHARD)
        biasr_np = np.ascontiguousarray(
            ((C0 - b2rot) / TSM).reshape(RJT, 128).T)     # [128, RJT]
        in_maps.append({
            "colg8": colg8_np,
            "colr8": colr8_np,
            "rhs8": rhs8_np,
            "biasr": biasr_np.astype(np.float32),
        })

    if _cached_nc is None:
        _cached_nc = _build_nc()
    res = run_bass_kernel_spmd(_cached_nc, in_maps,
                               core_ids=list(range(NCORES)),
                               trace=_trace)

    # ---- host combine ----
    NB = NCORES * NBLK                                    # 96 128-blocks
    sslot = np.array([jt for jt in range(NJT) if jt not in set(VSLOT)])
    vslot = np.array(VSLOT)
    # real: smooth-min partials -> coarse d^2 per (real, 512-block)
    d2s = np.full((N, NSB), np.inf, dtype=np.float32)
    d2v128 = np.full((N, NB), np.inf, dtype=np.float32)
    for c in range(NCORES):
        acc = np.stack([res.results[c][f"o_reals{k}"] for k in range(NIT)],
                       axis=-1)[:, sslot, :]
        with np.errstate(divide="ignore", invalid="ignore"):
            part = C0 - TSM * np.log(acc)                 # [128, nS, NIT]
        part = np.where(np.isfinite(part), part, np.inf).astype(np.float32)
        q = (c * SHARD + sslot[None, :, None] * JTILE
             + np.arange(128)[:, None, None]) % N
        sb = c * NIT + np.arange(NIT)[None, None, :]
        idx = (q * NSB + sb).ravel()
        np.minimum.at(d2s.ravel(), idx, part.ravel())
        # DVE-scanned real tiles: exact-ish 128-block partials of X
        rv = res.results[c]["o_realv"].reshape(128, NVR, NBLK)
        qv = (c * SHARD + vslot[None, :, None] * JTILE
              + np.arange(128)[:, None, None]) % N        # [128, NVR, 1]
        d2p = b2[qv] - rv                                 # d^2 partial
        gb = c * NBLK + np.arange(NBLK)[None, None, :]
        idxv = (qv * NB + gb).ravel()
        np.minimum.at(d2v128.ravel(), idxv, d2p.ravel())
    diag_sb = np.arange(N) // NT
    d2s_m = d2s.copy()
    d2s_m[np.arange(N), diag_sb] = np.inf                 # mask diag block
    d2v128[np.arange(N), np.arange(N) // JTILE] = np.inf  # mask diag block

    # gen: coarse block maxima of X = 2g.r - |r|^2
    Xb = np.empty((NB, N), dtype=np.float32)
    for c in range(NCORES):
        gv = res.results[c]["o_genv"].reshape(128, NJT, NBLK)
        Xb[c * NBLK:(c + 1) * NBLK, :] = (
            gv.transpose(2, 1, 0).reshape(NBLK, N))
    best = Xb.max(axis=0)
    cand_mask = Xb >= (best - MARGIN_G)[None, :]          # [96, N]
    Xstar = np.full(N, -np.inf, dtype=np.float32)
    istar = np.zeros(N, dtype=np.int64)
    for g in range(NB):
        js = np.nonzero(cand_mask[g])[0]
        if js.size == 0:
            continue
        rb = real[g * JTILE:(g + 1) * JTILE]              # [128, D]
        Xex = 2.0 * (gen[js] @ rb.T) - b2[g * JTILE:(g + 1) * JTILE][None, :]
        loc = np.argmax(Xex, axis=1)
        val = Xex[np.arange(js.size), loc]
        upd = val > Xstar[js]
        Xstar[js[upd]] = val[upd]
        istar[js[upd]] = g * JTILE + loc[upd]
    d1 = np.sqrt(np.maximum(a2g - Xstar, 0.0))

    # realNN: exact refinement only at the used indices
    used = np.unique(istar)
    du = d2s_m[used]                                      # [U, 24]
    duv = d2v128[used]                                    # [U, 96]
    coarse = np.minimum(du.min(axis=1), duv.min(axis=1))
    rcand = du <= (coarse + MARGIN_R)[:, None]
    rcand[~np.isfinite(coarse)] = True                    # fallback: all
    rcand[np.arange(used.size), diag_sb[used]] = True     # always diag
    rcandv = duv <= (coarse + MARGIN_RV)[:, None]         # [U, 96]
    nn2 = np.full(used.size, np.inf, dtype=np.float32)
    for g in range(NSB):
        rs = np.nonzero(rcand[:, g])[0]
        if rs.size == 0:
            continue
        ridx = used[rs]
        rb = real[g * NT:(g + 1) * NT]
        d2 = (b2[ridx][:, None] + b2[g * NT:(g + 1) * NT][None, :]
              - 2.0 * (real[ridx] @ rb.T))
        inblk = (ridx >= g * NT) & (ridx < (g + 1) * NT)
        d2[inblk, ridx[inblk] - g * NT] = np.inf          # exclude self
        nn2[rs] = np.minimum(nn2[rs], d2.min(axis=1))
    for g in range(NB):
        rs = np.nonzero(rcandv[:, g] & ~rcand[:, g // 4])[0]
        if rs.size == 0:
            continue
        ridx = used[rs]
        rb = real[g * JTILE:(g + 1) * JTILE]
        d2 = (b2[ridx][:, None] + b2[g * JTILE:(g + 1) * JTILE][None, :]
              - 2.0 * (real[ridx] @ rb.T))
        inblk = (ridx >= g * JTILE) & (ridx < (g + 1) * JTILE)
        d2[inblk, ridx[inblk] - g * JTILE] = np.inf       # exclude self
        nn2[rs] = np.minimum(nn2[rs], d2.min(axis=1))
    lut = np.zeros(N, dtype=np.float32)
    lut[used] = np.sqrt(np.maximum(nn2, 0.0))
    d2v = lut[istar]

    z = (d2v - d1) / 0.1
    authen = np.where(z >= 0, 1.0 / (1.0 + np.exp(-np.abs(z))),
                      np.exp(-np.abs(z)) / (1.0 + np.exp(-np.abs(z))))
    out = np.asarray(-100.0 * np.mean(authen), dtype=np.float32)
    if _trace:
        return out, res
    return out

